# revision 1
# baseline (speedup 1.0000x reference)
"""CGRUCell Trainium2 kernel — hybrid: data-parallel x4 over batch,
tensor-parallel x2 over units, on 8 NeuronCores.

Core c: batch group g=c//2 (256 rows), unit parity p=c%2 (units
[p*1024:(p+1)*1024], i.e. its real+imag output columns). Weights are
split by parity (100.7 MB/core instead of 201 MB replicated). The only
cross-core data is r*h for the candidate gate: a pairwise AllGather,
overlapped with gate-z matmuls.

Gate r is computed output-transposed (weights stationary, activations
moving) so r and r*h are produced directly in K-major layout — no
on-chip transposes. Gates z and h use the batch-stationary orientation
(N=512 moving) for better weight-load amortization.

Matmuls run in float32r (TF32-like full-rate fp32 mode, rel err ~1e-4).
Gate order: r -> z -> h so the r*h exchange hides under z's compute.
"""

import sys

for _p in ("/opt/trn_rl_repo", "/root/.axon_site/_ro/trn_rl_repo"):
    if _p not in sys.path:
        sys.path.append(_p)

import numpy as np

import concourse.bass as bass
import concourse.mybir as mybir
import concourse.tile as tile
from concourse import bacc
from concourse.bass_utils import run_bass_kernel_spmd

P = 128
U = 2048           # UNITS
B = 1024
N_CORES = 8
GROUPS = 4
BC = B // GROUPS   # 256 batch rows per core
MSUB = BC // P     # 2 m-subtiles
UC = U // 2        # 1024 unit columns per core (per half)
KCH = U // P       # 16 k-chunks per complex half
F32 = mybir.dt.float32
MM_DT = mybir.dt.float32r
NBLK = UC // 512   # 2 col-blocks of 512 per half
BLK = 512

_CACHE = {}


def _build_nc(repeat=1):
    nc = bacc.Bacc(None, target_bir_lowering=False)

    # K-major activations (full contraction dims, batch 256 of this group)
    x1 = nc.dram_tensor("x1", [P, KCH, BC], MM_DT, kind="ExternalInput")
    x2 = nc.dram_tensor("x2", [P, KCH, BC], MM_DT, kind="ExternalInput")
    h1 = nc.dram_tensor("h1", [P, KCH, BC], MM_DT, kind="ExternalInput")
    h2 = nc.dram_tensor("h2", [P, KCH, BC], MM_DT, kind="ExternalInput")
    # batch-major h_tm1, own columns only: [256, 2048]
    hbm = nc.dram_tensor("hbm", [BC, 2 * UC], F32, kind="ExternalInput")
    # K-major h_tm1, own columns: [p, o(8 re + 8 im), b]
    hTo = nc.dram_tensor("hTo", [P, KCH, BC], MM_DT, kind="ExternalInput")
    # weights, parity-sliced on host: [2048, 3*1024] (gate z|r|h blocks)
    RK = nc.dram_tensor("RK", [U, 3 * UC], MM_DT, kind="ExternalInput")
    IK = nc.dram_tensor("IK", [U, 3 * UC], MM_DT, kind="ExternalInput")
    RR = nc.dram_tensor("RR", [U, 3 * UC], MM_DT, kind="ExternalInput")
    IR = nc.dram_tensor("IR", [U, 3 * UC], MM_DT, kind="ExternalInput")
    # bias for z/h, own columns, block order [gate(3), half(2), 1024]
    # (gate r entries unused, kept for layout simplicity)
    bias = nc.dram_tensor("bias", [3 * 2 * UC], F32, kind="ExternalInput")
    # gate-r bias, column-major pre-scaled: b' = 0.2*b + 0.5, [128, 16]
    biasr = nc.dram_tensor("biasr", [P, KCH], F32, kind="ExternalInput")
    out = nc.dram_tensor("out", [BC, 2 * UC], F32, kind="ExternalOutput")

    hbm_r = hbm.rearrange("(m p) c -> p m c", p=P)
    out_r = out.rearrange("(m p) c -> p m c", p=P)

    with tile.TileContext(nc) as tc:
        with (
            tc.tile_pool(name="acts", bufs=1) as acts,
            tc.tile_pool(name="wpool", bufs=2) as wpool,
            tc.tile_pool(name="psum", bufs=1, space="PSUM") as psum,
            tc.tile_pool(name="small", bufs=2) as small,
            tc.tile_pool(name="neg", bufs=2) as negp,
            tc.tile_pool(name="bigs", bufs=1) as bigs,
            tc.tile_pool(name="dram", bufs=1, space="DRAM") as dram,
        ):
            x1s = acts.tile([P, KCH, BC], MM_DT, tag="x1s", name="x1s")
            x2s = acts.tile([P, KCH, BC], MM_DT, tag="x2s", name="x2s")
            h1s = acts.tile([P, KCH, BC], MM_DT, tag="h1s", name="h1s")
            h2s = acts.tile([P, KCH, BC], MM_DT, tag="h2s", name="h2s")
            hTos = acts.tile([P, KCH, BC], MM_DT, tag="hTos", name="hTos")
            rh2s = acts.tile([P, KCH, BC], MM_DT, tag="rh2s", name="rh2s")
            # chunked act loads: the first k-slices land in ~2us so the
            # gate-r matmuls start without waiting for the full 10 MB
            for o in range(0, KCH, 4):
                sl = slice(o, o + 4)
                nc.sync.dma_start(x1s[:, sl, :], x1[:, sl, :])
                nc.sync.dma_start(x2s[:, sl, :], x2[:, sl, :])
                nc.sync.dma_start(h1s[:, sl, :], h1[:, sl, :])
                nc.sync.dma_start(h2s[:, sl, :], h2[:, sl, :])
                nc.sync.dma_start(hTos[:, sl, :], hTo[:, sl, :])

            hbmt = bigs.tile([P, MSUB, 2 * UC], F32, tag="hbmt", name="hbmt")
            nc.sync.dma_start(hbmt[:], hbm_r)

            z_sb = bigs.tile([P, MSUB, 2 * UC], F32, tag="z_sb", name="z_sb")
            hh_sb = bigs.tile([P, MSUB, 2 * UC], F32, tag="hh_sb", name="hh_sb")

            brcol = small.tile([P, KCH], F32, tag="brcol", name="brcol", bufs=1)
            nc.sync.dma_start(brcol[:], biasr[:])

            def wtile(rep, g, tname, k, bb, src, width=BLK):
                t = wpool.tile([P, BLK], MM_DT, tag=tname,
                               name=f"{tname}_{rep}_{g}_{k}_{bb}")
                ksl = slice(k * P, (k + 1) * P)
                csl = slice(g * UC + bb * width, g * UC + (bb + 1) * width)
                nc.sync.dma_start(t[:], src[ksl, csl])
                return t

            def gate_phase_a(rep, g, b1, b2, evict):
                """Batch-stationary gates (z, h): 8 psum blocks
                [m, half, bb] of [128 batch, 512 cols]."""
                ps = {}
                for m in range(MSUB):
                    for half in range(2):
                        for bb in range(NBLK):
                            ps[(m, half, bb)] = psum.tile(
                                [P, BLK], F32,
                                tag=f"ps{(m * 2 + half) * NBLK + bb}",
                                name=f"ps_{rep}_{g}_{m}_{half}_{bb}",
                            )
                for k in range(KCH):
                    wts = {
                        n: [wtile(rep, g, f"{n}{bb}", k, bb, src)
                            for bb in range(NBLK)]
                        for n, src in (("rk", RK), ("ik", IK),
                                       ("rr", RR), ("ir", IR))
                    }
                    na1 = negp.tile([P, BC], MM_DT, tag="na1",
                                    name=f"na1_{rep}_{g}_{k}")
                    nb1 = negp.tile([P, BC], MM_DT, tag="nb1",
                                    name=f"nb1_{rep}_{g}_{k}")
                    nc.vector.tensor_scalar(
                        na1[:], x1s[:, k, :], -1.0, None, mybir.AluOpType.mult
                    )
                    nc.vector.tensor_scalar(
                        nb1[:], b1[:, k, :], -1.0, None, mybir.AluOpType.mult
                    )

                    first = k == 0
                    last = k == KCH - 1
                    streams = [
                        (x1s, "rk", 0, first, False),
                        (x2s, "rk", 1, first, False),
                        (x2s, "ik", 0, False, False),
                        (na1, "ik", 1, False, False),
                        (b1, "rr", 0, False, False),
                        (b2, "rr", 1, False, False),
                        (b2, "ir", 0, False, last),
                        (nb1, "ir", 1, False, last),
                    ]
                    for stat, wn, half, st, sp in streams:
                        for m in range(MSUB):
                            if stat is na1 or stat is nb1:
                                lhsT = stat[:, m * P : (m + 1) * P]
                            else:
                                lhsT = stat[:, k, m * P : (m + 1) * P]
                            for bb in range(NBLK):
                                nc.tensor.matmul(
                                    ps[(m, half, bb)],
                                    lhsT,
                                    wts[wn][bb][:],
                                    start=st,
                                    stop=sp,
                                )

                for half in range(2):
                    for bb in range(NBLK):
                        j = half * NBLK + bb
                        bt = small.tile([P, BLK], F32, tag="bt",
                                        name=f"bt_{rep}_{g}_{j}")
                        nc.sync.dma_start(
                            bt[:],
                            bias[None, g * 2 * UC + j * BLK : g * 2 * UC
                                 + (j + 1) * BLK].to_broadcast((P, BLK)),
                        )
                        for m in range(MSUB):
                            oc = slice(half * UC + bb * BLK,
                                       half * UC + (bb + 1) * BLK)
                            evict(ps[(m, half, bb)], bt, m, oc)

            def evict_hs(dest):
                def _e(pst, bt, m, oc):
                    d = dest[:, m, oc]
                    nc.vector.tensor_add(d, pst[:], bt[:])
                    nc.vector.tensor_scalar(
                        d, d, 0.2, 0.5,
                        mybir.AluOpType.mult, mybir.AluOpType.add,
                    )
                    nc.vector.tensor_scalar(
                        d, d, 1.0, 0.0,
                        mybir.AluOpType.min, mybir.AluOpType.max,
                    )
                return _e

            def evict_tanh(dest):
                def _e(pst, bt, m, oc):
                    tmp = small.tile([P, BLK], F32, tag="ttmp", name="ttmp")
                    nc.vector.tensor_add(tmp[:], pst[:], bt[:])
                    nc.scalar.activation(
                        dest[:, m, oc], tmp[:], mybir.ActivationFunctionType.Tanh
                    )
                return _e

            def gate_r_transposed(rep, rhTl):
                """Gate r, output-transposed: psum [128 cols, 256 batch].
                grp 0 covers weight cols 0:512 (real ccs 0-3 + imag ccs 0-3),
                grp 1 covers cols 512:1024. Writes rhT = hs(pre_r)*hT
                directly into rhTl[:, ccg, :]."""
                g = 1
                for grp in range(2):
                    ps = [
                        psum.tile([P, BC], F32, tag=f"ps{i}",
                                  name=f"psr_{rep}_{grp}_{i}")
                        for i in range(8)
                    ]
                    # ps[0..3] real ccs, ps[4..7] imag ccs
                    for k in range(KCH):
                        wts = {
                            n: wtile(rep, g, f"{n}{grp}", k, grp, src)
                            for n, src in (("rk", RK), ("ik", IK),
                                           ("rr", RR), ("ir", IR))
                        }
                        na1 = negp.tile([P, BC], MM_DT, tag="na1",
                                        name=f"na1r_{rep}_{grp}_{k}")
                        nb1 = negp.tile([P, BC], MM_DT, tag="nb1",
                                        name=f"nb1r_{rep}_{grp}_{k}")
                        nc.vector.tensor_scalar(
                            na1[:], x1s[:, k, :], -1.0, None,
                            mybir.AluOpType.mult,
                        )
                        nc.vector.tensor_scalar(
                            nb1[:], h1s[:, k, :], -1.0, None,
                            mybir.AluOpType.mult,
                        )
                        first = k == 0
                        last = k == KCH - 1
                        # (weight, moving, psum base, start, stop)
                        streams = [
                            ("rk", x1s, 0, first, False),
                            ("rk", x2s, 4, first, False),
                            ("ik", x2s, 0, False, False),
                            ("ik", na1, 4, False, False),
                            ("rr", h1s, 0, False, False),
                            ("rr", h2s, 4, False, False),
                            ("ir", h2s, 0, False, last),
                            ("ir", nb1, 4, False, last),
                        ]
                        for wn, mov, base, st, sp in streams:
                            if mov is na1 or mov is nb1:
                                rhs = mov[:]
                            else:
                                rhs = mov[:, k, :]
                            for cc in range(4):
                                nc.tensor.matmul(
                                    ps[base + cc],
                                    wts[wn][:, cc * P : (cc + 1) * P],
                                    rhs,
                                    start=st,
                                    stop=sp,
                                )
                    # evict: rhT[ccg] = clip(0.2*psum + b') * hT_own[ccg]
                    for i in range(8):
                        half = i // 4          # 0 real, 1 imag
                        ccg = half * 8 + grp * 4 + (i % 4)
                        d = rhTl[:, ccg, :]
                        nc.vector.tensor_scalar(
                            d, ps[i][:], 0.2, brcol[:, ccg : ccg + 1],
                            mybir.AluOpType.mult, mybir.AluOpType.add,
                        )
                        nc.vector.tensor_scalar(
                            d, d, 1.0, 0.0,
                            mybir.AluOpType.min, mybir.AluOpType.max,
                        )
                        nc.vector.tensor_mul(d, d, hTos[:, ccg, :])

            for rep in range(repeat):
                # --- gate r first (g=1), output-transposed ---
                rhTl = acts.tile([P, KCH, BC], MM_DT, tag="rh1s",
                                 name=f"rhTl_{rep}")
                gate_r_transposed(rep, rhTl)

                # pairwise AllGather of rhT
                inb = dram.tile([P, KCH, BC], MM_DT, tag="inb",
                                name=f"inb_{rep}")
                outb = dram.tile([2, P, KCH, BC], MM_DT, tag="outb",
                                 name=f"outb_{rep}")
                nc.sync.dma_start(inb[:], rhTl[:])
                nc.gpsimd.collective_compute(
                    "AllGather",
                    mybir.AluOpType.bypass,
                    replica_groups=[[0, 1], [2, 3], [4, 5], [6, 7]],
                    ins=[inb[:].opt()],
                    outs=[outb[:].opt()],
                )
                rh1s = acts.tile([P, KCH, BC], MM_DT, tag="rh1s",
                                 name=f"rh1s_{rep}")
                # real rows: parity0 units 0:1024 -> o 0..7, parity1 -> 8..15
                nc.sync.dma_start(rh1s[:, 0:8, :], outb[0, :, 0:8, :])
                nc.sync.dma_start(rh1s[:, 8:16, :], outb[1, :, 0:8, :])
                nc.sync.dma_start(rh2s[:, 0:8, :], outb[0, :, 8:16, :])
                nc.sync.dma_start(rh2s[:, 8:16, :], outb[1, :, 8:16, :])

                # --- gate z (g=0), overlaps with the collective ---
                gate_phase_a(rep, 0, h1s, h2s, evict_hs(z_sb))

                # --- gate h (g=2) ---
                gate_phase_a(rep, 2, rh1s, rh2s, evict_tanh(hh_sb))

                # h_new = hh + z*(h - hh), in place into hbmt;
                # per m-subtile so DVE of one overlaps the out-DMA of the other
                for m in range(MSUB):
                    nc.vector.tensor_sub(
                        hbmt[:, m, :], hbmt[:, m, :], hh_sb[:, m, :]
                    )
                    nc.vector.tensor_mul(
                        hbmt[:, m, :], z_sb[:, m, :], hbmt[:, m, :]
                    )
                    nc.vector.tensor_add(
                        hbmt[:, m, :], hh_sb[:, m, :], hbmt[:, m, :]
                    )
                    nc.sync.dma_start(out_r[:, m, :], hbmt[:, m, :])

    nc.compile()
    return nc


def _pack_kmajor(a):
    # (BC, 2048) -> (128, 16, BC) with [p, o, b] = a[b, o*128+p]
    bc = a.shape[0]
    return np.ascontiguousarray(a.T.reshape(KCH, P, bc).transpose(1, 0, 2))


def make_in_maps(
    inputs, h_tm1, real_kernel, imaginary_kernel,
    real_recurrent_kernel, imaginary_recurrent_kernel, real_bias,
    imaginary_bias,
):
    inputs = np.ascontiguousarray(inputs, dtype=np.float32)
    h_tm1 = np.ascontiguousarray(h_tm1, dtype=np.float32)
    ws = {
        "RK": np.ascontiguousarray(real_kernel, dtype=np.float32),
        "IK": np.ascontiguousarray(imaginary_kernel, dtype=np.float32),
        "RR": np.ascontiguousarray(real_recurrent_kernel, dtype=np.float32),
        "IR": np.ascontiguousarray(imaginary_recurrent_kernel, dtype=np.float32),
    }
    rb = np.asarray(real_bias, dtype=np.float32)
    ib = np.asarray(imaginary_bias, dtype=np.float32)

    wsl = {}
    bsl = {}
    brc = {}
    for p in range(2):
        cols = [slice(g * U + p * UC, g * U + (p + 1) * UC) for g in range(3)]
        wsl[p] = {
            k: np.ascontiguousarray(np.concatenate([v[:, c] for c in cols], axis=1))
            for k, v in ws.items()
        }
        bsl[p] = np.concatenate([np.concatenate([rb[c], ib[c]]) for c in cols])
        # gate-r column-major bias, pre-scaled: [128, 16], [pp, ccg]
        br = np.concatenate([rb[cols[1]], ib[cols[1]]])  # (2048,) re|im own
        brc[p] = np.ascontiguousarray(
            (0.2 * br + 0.5).reshape(KCH, P).T
        )

    in_maps = []
    for c in range(N_CORES):
        g, p = c // 2, c % 2
        rs = slice(g * BC, (g + 1) * BC)
        ocr = slice(p * UC, (p + 1) * UC)
        oci = slice(U + p * UC, U + (p + 1) * UC)
        hbm = np.ascontiguousarray(
            np.concatenate([h_tm1[rs, ocr], h_tm1[rs, oci]], axis=1)
        )
        in_maps.append(
            {
                "x1": _pack_kmajor(inputs[rs, :U]),
                "x2": _pack_kmajor(inputs[rs, U:]),
                "h1": _pack_kmajor(h_tm1[rs, :U]),
                "h2": _pack_kmajor(h_tm1[rs, U:]),
                "hbm": hbm,
                "hTo": _pack_kmajor(hbm),
                "RK": wsl[p]["RK"],
                "IK": wsl[p]["IK"],
                "RR": wsl[p]["RR"],
                "IR": wsl[p]["IR"],
                "bias": bsl[p],
                "biasr": brc[p],
            }
        )
    return in_maps


def scatter_out(results):
    h_new = np.empty((B, 2 * U), dtype=np.float32)
    for c in range(N_CORES):
        g, p = c // 2, c % 2
        rs = slice(g * BC, (g + 1) * BC)
        o = results[c]["out"]
        h_new[rs, p * UC : (p + 1) * UC] = o[:, :UC]
        h_new[rs, U + p * UC : U + (p + 1) * UC] = o[:, UC:]
    return h_new


def _build_nc_retry(repeat=1, attempts=4):
    # Tile's scheduler very occasionally reports a spurious deadlock on a
    # valid graph (ordering is not fully deterministic); retry a few times.
    last = None
    for _ in range(attempts):
        try:
            return _build_nc(repeat=repeat)
        except Exception as e:  # noqa: BLE001
            if "Deadlock" not in type(e).__name__ + str(e):
                raise
            last = e
    raise last


def kernel(
    inputs,
    h_tm1,
    real_kernel,
    imaginary_kernel,
    real_recurrent_kernel,
    imaginary_recurrent_kernel,
    real_bias,
    imaginary_bias,
):
    if "nc" not in _CACHE:
        _CACHE["nc"] = _build_nc_retry()
    nc = _CACHE["nc"]
    in_maps = make_in_maps(
        inputs, h_tm1, real_kernel, imaginary_kernel,
        real_recurrent_kernel, imaginary_recurrent_kernel, real_bias,
        imaginary_bias,
    )
    res = run_bass_kernel_spmd(nc, in_maps, core_ids=list(range(N_CORES)))
    return scatter_out(res.results)

